# revision 9
# baseline (speedup 1.0000x reference)
"""GCN (3x GCNConv+BN+ReLU -> 2-layer MLP) on 8 Trainium2 NeuronCores.

Strategy (graph/data parallel, hardcoded for N=50000, E=800000):
  - Nodes sharded 6250/core.  Edges owned by dst core.  Self-loop terms are
    appended as explicit edges (coef = dinv^2) on the host.
  - Per layer: each core computes t = h @ W rows for its own nodes (PE),
    AllGather replicates the full [50000,128] table into every core's DRAM,
    then each core gathers t[src] rows for its edges with SWDGE dma_gather
    (int16 indices -> the table is addressed in two 25000-row halves),
    and scatter-adds into 125-node windows with a one-hot matmul trick:
        onehot[e, n] = (iota[n] == dst_local[e]) * coef[e]   (one DVE op)
        psum[f, n]  += gathered[e, f].T @ onehot[e, n]        (one PE matmul)
    BN+ReLU is fused into the PSUM eviction as a single ScalarE activation
    (per-feature scale/bias live on partitions).
  - Final MLP runs on local shards; output rows DMA'd out and concatenated
    on the host.

All graph preprocessing (degree, normalization coefs, edge sort/padding,
index layout for dma_gather) happens on the host in numpy; the edge
schedule is padded to be identical across all 8 cores (SPMD: one program).
"""

import math
import os

import numpy as np

import concourse.bass as bass
import concourse.bacc as bacc
import concourse.mybir as mybir
from concourse.tile import TileContext, add_dep_helper
from concourse.bass_utils import run_bass_kernel_spmd

# ----- problem constants (hardcoded; must match the grading problem) -----
N = 50000
E = 800000
IN_DIM = 64
HID = 128
OUT = 64
EPS = 1e-5
NCORES = 8
WIN = 125            # nodes per scatter window (PSUM free dim)
CHUNK = 1024         # edges per dma_gather call (8 groups of 128);
                     # 2048-idx dma_gather calls crash the exec unit
P = 128              # partitions / edge-group size

F32 = mybir.dt.float32
I16 = mybir.dt.int16


def _derived():
    SH = N // NCORES
    NW = SH // WIN
    HALF = N // 2
    assert SH * NCORES == N and NW * WIN == SH and HALF * 2 == N
    return SH, NW, HALF


def _row_of_node(n):
    """DRAM table row of node n (table stores shard tiles in [p, i, f] order:
    local l = i*WIN + p  ->  row = rank*SH + p*NW + i)."""
    SH, NW, _ = _derived()
    r = n // SH
    l = n % SH
    return r * SH + (l % WIN) * NW + l // WIN


def _preprocess(src, dst):
    """Build the (identical-across-cores) edge schedule + per-core data."""
    SH, NW, HALF = _derived()
    src = np.asarray(src).astype(np.int64)
    dst = np.asarray(dst).astype(np.int64)

    deg = 1.0 + np.bincount(dst, minlength=N).astype(np.float64)
    dinv = (1.0 / np.sqrt(deg)).astype(np.float32)

    per_core = []
    counts = np.zeros((NCORES, NW, 2), np.int64)
    for c in range(NCORES):
        sel = (dst // SH) == c
        es = src[sel]
        ed = dst[sel]
        coef = (dinv[es] * dinv[ed]).astype(np.float32)
        nodes = np.arange(c * SH, (c + 1) * SH, dtype=np.int64)
        es = np.concatenate([es, nodes])
        ed = np.concatenate([ed, nodes])
        coef = np.concatenate([coef, (dinv[nodes] * dinv[nodes]).astype(np.float32)])
        l = ed - c * SH
        win = l // WIN
        nloc = l % WIN
        half = es // HALF
        order = np.lexsort((half, win))
        es, coef, win, nloc, half = (a[order] for a in (es, coef, win, nloc, half))
        np.add.at(counts[c], (win, half), 1)
        per_core.append((es, coef, win, nloc, half))

    # groups per (window, half): max over cores, in units of 128 edges
    G = np.ceil(counts.max(axis=0) / P).astype(np.int64)  # [NW, 2]
    G = np.maximum(G, 1)

    # per-half stream layouts (same for every core)
    stream_len = [int(G[:, s].sum()) * P for s in range(2)]
    n_groups = [int(G[:, s].sum()) for s in range(2)]
    # group j of stream s targets window gw[s][j]
    gw = [np.repeat(np.arange(NW), G[:, s]) for s in range(2)]

    row_of = _row_of_node(np.arange(N, dtype=np.int64))

    data = []
    for c in range(NCORES):
        es, coef, win, nloc, half = per_core[c]
        rows = row_of[es]
        streams = []
        for s in range(2):
            L = stream_len[s]
            idx_local = np.zeros(L, np.int64)
            co = np.zeros(L, np.float32)
            dl = np.zeros(L, np.float32)
            pos = 0
            for w in range(NW):
                m = (win == w) & (half == s)
                k = int(m.sum())
                cap = int(G[w, s]) * P
                assert k <= cap
                idx_local[pos:pos + k] = rows[m] - s * HALF
                co[pos:pos + k] = coef[m]
                dl[pos:pos + k] = nloc[m].astype(np.float32)
                # padding: idx 0 (valid row), coef 0 -> contributes nothing
                pos += cap
            assert pos == L
            assert idx_local.min() >= 0 and idx_local.max() < HALF <= 32768
            # idx tile [128, L/16]: chunk-local wrap (p = j%16, col = j//16),
            # replicated across the 8 groups of 16 partitions
            A = np.zeros((P, L // 16), np.int16)
            pos = 0
            while pos < L:
                cn = min(CHUNK, L - pos)
                blk = idx_local[pos:pos + cn].reshape(cn // 16, 16).T  # [16, cn/16]
                c0 = pos // 16
                for r in range(8):
                    A[r * 16:(r + 1) * 16, c0:c0 + cn // 16] = blk
                pos += cn
            ng = n_groups[s]
            dl_t = dl.reshape(ng, P).T.copy()          # [128, ng]
            co_t = co.reshape(ng, P).T.copy()          # [128, ng]
            streams.append((A, dl_t, co_t))
        data.append(streams)

    sched = {
        "G": G, "gw": gw, "stream_len": stream_len, "n_groups": n_groups,
        "dinv": dinv,
    }
    return sched, data


def _chunks_of(L):
    out = []
    pos = 0
    while pos < L:
        cn = min(CHUNK, L - pos)
        out.append((pos, cn))
        pos += cn
    return out


def _build_program(sched):
    SH, NW, HALF = _derived()
    G = sched["G"]
    stream_len = sched["stream_len"]
    n_groups = sched["n_groups"]

    nc = bacc.Bacc(trn_type="TRN2", num_devices=NCORES)
    f = F32

    # ---- I/O ----
    xT_d = nc.dram_tensor("xT", [IN_DIM, SH], f, kind="ExternalInput")
    W_d = [
        nc.dram_tensor("W0", [IN_DIM, HID], f, kind="ExternalInput"),
        nc.dram_tensor("W1", [HID, HID], f, kind="ExternalInput"),
        nc.dram_tensor("W2", [HID, HID], f, kind="ExternalInput"),
    ]
    S_d = [nc.dram_tensor(f"S{i}", [HID, 1], f, kind="ExternalInput") for i in range(3)]
    T_d = [nc.dram_tensor(f"T{i}", [HID, 1], f, kind="ExternalInput") for i in range(3)]
    Wm1_d = nc.dram_tensor("Wm1", [HID, HID], f, kind="ExternalInput")
    Wm2_d = nc.dram_tensor("Wm2", [HID, OUT], f, kind="ExternalInput")
    bm1_d = nc.dram_tensor("bm1", [HID, 1], f, kind="ExternalInput")
    bm2b_d = nc.dram_tensor("bm2b", [P, OUT], f, kind="ExternalInput")
    iota_d = nc.dram_tensor("iota", [P, WIN], f, kind="ExternalInput")
    idx_d = [nc.dram_tensor(f"idx{s}", [P, stream_len[s] // 16], I16,
                            kind="ExternalInput") for s in range(2)]
    dstl_d = [nc.dram_tensor(f"dstl{s}", [P, n_groups[s]], f,
                             kind="ExternalInput") for s in range(2)]
    coef_d = [nc.dram_tensor(f"coef{s}", [P, n_groups[s]], f,
                             kind="ExternalInput") for s in range(2)]

    shard_d = nc.dram_tensor("shard", [SH, HID], f, kind="Internal")
    tab_d = [nc.dram_tensor(f"tab{i}", [N, HID], f, kind="Internal")
             for i in range(2)]
    out_d = nc.dram_tensor("out", [SH, OUT], f, kind="ExternalOutput")

    rg = [list(range(NCORES))]
    relu = mybir.ActivationFunctionType.Relu
    copyf = mybir.ActivationFunctionType.Copy

    with TileContext(nc) as tc:
        with (
            tc.tile_pool(name="const", bufs=1) as constp,
            tc.tile_pool(name="hTp", bufs=2) as hTp,
            tc.tile_pool(name="g0p", bufs=3) as g0p,
            tc.tile_pool(name="g1p", bufs=3) as g1p,
            tc.tile_pool(name="ohp", bufs=6) as ohp,
            tc.tile_pool(name="rowp", bufs=4) as rowp,
            tc.tile_pool(name="psA", bufs=2, space="PSUM") as psA,
            tc.tile_pool(name="psB", bufs=2, space="PSUM") as psB,
        ):
            # ---- resident constants ----
            xT_sb = constp.tile([IN_DIM, SH], f, name="xT_sb")
            nc.sync.dma_start(xT_sb[:], xT_d[:])
            W_sb = []
            for i in range(3):
                t = constp.tile([W_d[i].shape[0], HID], f, name=f"W{i}_sb")
                nc.sync.dma_start(t[:], W_d[i][:])
                W_sb.append(t)
            Wm1_sb = constp.tile([HID, HID], f, name="Wm1_sb")
            nc.sync.dma_start(Wm1_sb[:], Wm1_d[:])
            Wm2_sb = constp.tile([HID, OUT], f, name="Wm2_sb")
            nc.sync.dma_start(Wm2_sb[:], Wm2_d[:])
            bm1_sb = constp.tile([HID, 1], f, name="bm1_sb")
            nc.sync.dma_start(bm1_sb[:], bm1_d[:])
            bm2b_sb = constp.tile([P, OUT], f, name="bm2b_sb")
            nc.sync.dma_start(bm2b_sb[:], bm2b_d[:])
            iota_sb = constp.tile([P, WIN], f, name="iota_sb")
            nc.sync.dma_start(iota_sb[:], iota_d[:])
            S_sb, T_sb = [], []
            for i in range(3):
                ts_ = constp.tile([HID, 1], f, name=f"S{i}_sb")
                nc.sync.dma_start(ts_[:], S_d[i][:])
                S_sb.append(ts_)
                tt_ = constp.tile([HID, 1], f, name=f"T{i}_sb")
                nc.sync.dma_start(tt_[:], T_d[i][:])
                T_sb.append(tt_)
            idx_sb, dstl_sb, coef_sb = [], [], []
            for s in range(2):
                ti = constp.tile([P, stream_len[s] // 16], I16, name=f"idx{s}_sb")
                nc.sync.dma_start(ti[:], idx_d[s][:])
                idx_sb.append(ti)
                td = constp.tile([P, n_groups[s]], f, name=f"dstl{s}_sb")
                nc.sync.dma_start(td[:], dstl_d[s][:])
                dstl_sb.append(td)
                tcf = constp.tile([P, n_groups[s]], f, name=f"coef{s}_sb")
                nc.sync.dma_start(tcf[:], coef_d[s][:])
                coef_sb.append(tcf)

            shard_v = shard_d[:].rearrange("(p i) a -> p i a", i=NW)  # [125,50,128]

            hT = None  # [128, SH] activations, transposed (feat on partitions)
            for layer in range(3):
                # ---- phase 1: shard rows of t = h @ W (node-tile i holds
                #      locals l = i*WIN + p; table row = p*NW + i) ----
                for i in range(NW):
                    if layer == 0:
                        lhsT = xT_sb[:, i * WIN:(i + 1) * WIN]
                    else:
                        lhsT = hT[:, i * WIN:(i + 1) * WIN]
                    ps = psB.tile([WIN, HID], f, tag="ph1", name="ps_ph1")
                    nc.tensor.matmul(ps[:], lhsT, W_sb[layer][:],
                                     start=True, stop=True)
                    row_sb = rowp.tile([WIN, HID], f, tag="row", name="row_sb")
                    nc.scalar.activation(row_sb[:], ps[:], copyf)
                    nc.sync.dma_start(shard_v[:, i, :], row_sb[:])

                tab = tab_d[layer % 2]
                cc = nc.gpsimd.collective_compute(
                    "AllGather", mybir.AluOpType.bypass, replica_groups=rg,
                    ins=[shard_d[:]], outs=[tab[:]],
                )

                # ---- phase 2: gather + one-hot scatter matmul ----
                hT_new = hTp.tile([HID, SH], f, tag="hT", name="hT_new")
                cur_chunk = [-1, -1]
                chunk_tile = [None, None]
                gpools = [g0p, g1p]
                gbase = [0, 0]  # group index base per stream
                chunk_list = [_chunks_of(stream_len[s]) for s in range(2)]

                for w in range(NW):
                    ps = psA.tile([HID, WIN], f, tag="agg", name="ps_agg")
                    ngw = int(G[w, 0] + G[w, 1])
                    k = 0
                    for s in range(2):
                        for j in range(gbase[s], gbase[s] + int(G[w, s])):
                            ck = (j * P) // CHUNK
                            if cur_chunk[s] != ck:
                                pos, cn = chunk_list[s][ck]
                                gt = gpools[s].tile(
                                    [P, cn // P, HID], f, tag=f"g{s}",
                                    name=f"gt{s}",
                                )
                                gi = nc.gpsimd.dma_gather(
                                    gt[:],
                                    tab[s * HALF:(s + 1) * HALF, :],
                                    idx_sb[s][:, pos // 16:(pos + cn) // 16],
                                    cn, cn, HID,
                                )
                                add_dep_helper(gi.ins, cc.ins, True,
                                               "gather after allgather")
                                chunk_tile[s] = gt
                                cur_chunk[s] = ck
                            g_ap = chunk_tile[s][:, (j * P % CHUNK) // P, :]
                            oh = ohp.tile([P, WIN], f, tag="oh", name="oh")
                            nc.vector.tensor_scalar(
                                oh[:], iota_sb[:],
                                dstl_sb[s][:, j:j + 1],
                                coef_sb[s][:, j:j + 1],
                                mybir.AluOpType.is_equal,
                                mybir.AluOpType.mult,
                            )
                            nc.tensor.matmul(ps[:], g_ap, oh[:],
                                             start=(k == 0), stop=(k == ngw - 1))
                            k += 1
                    gbase[0] += int(G[w, 0])
                    gbase[1] += int(G[w, 1])
                    # fused BN+ReLU eviction
                    nc.scalar.activation(
                        hT_new[:, w * WIN:(w + 1) * WIN], ps[:], relu,
                        bias=T_sb[layer][:], scale=S_sb[layer][:],
                    )
                # reset per-layer chunk state
                hT = hT_new

            # ---- MLP ----
            z1T = hTp.tile([HID, SH], f, tag="hT", name="z1T")
            pos = 0
            while pos < SH:
                cn = min(512, SH - pos)
                ps = psB.tile([HID, 512], f, tag="mlp1", name="ps_mlp1")
                nc.tensor.matmul(ps[:, :cn], Wm1_sb[:], hT[:, pos:pos + cn],
                                 start=True, stop=True)
                nc.scalar.activation(z1T[:, pos:pos + cn], ps[:, :cn], relu,
                                     bias=bm1_sb[:])
                pos += cn
            for i in range(NW):
                ps2 = psB.tile([WIN, OUT], f, tag="mlp2", name="ps_mlp2")
                nc.tensor.matmul(ps2[:], z1T[:, i * WIN:(i + 1) * WIN],
                                 Wm2_sb[:], start=True, stop=True)
                ot = rowp.tile([WIN, OUT], f, tag="ot", name="ot")
                nc.vector.tensor_tensor(ot[:], ps2[:], bm2b_sb[:WIN, :],
                                        op=mybir.AluOpType.add)
                nc.sync.dma_start(out_d[i * WIN:(i + 1) * WIN, :], ot[:])

    return nc


def _build_inputs(inputs, sched, data):
    """Per-core in_maps."""
    SH, NW, HALF = _derived()
    x = np.asarray(inputs["x"], np.float32)

    def bnfold(g, be, m, v, b):
        S = (np.asarray(g, np.float32)
             / np.sqrt(np.asarray(v, np.float32) + np.float32(EPS)))
        T = np.asarray(be, np.float32) + (np.asarray(b, np.float32)
                                          - np.asarray(m, np.float32)) * S
        return S.astype(np.float32), T.astype(np.float32)

    S0, T0 = bnfold(inputs["g0"], inputs["be0"], inputs["m0"], inputs["v0"], inputs["b0"])
    S1, T1 = bnfold(inputs["g1"], inputs["be1"], inputs["m1"], inputs["v1"], inputs["b1"])
    S2, T2 = bnfold(inputs["g2"], inputs["be2"], inputs["m2"], inputs["v2"], inputs["b2"])
    Ss = [S0, S1, S2]
    Ts = [T0, T1, T2]

    iota = np.broadcast_to(np.arange(WIN, dtype=np.float32)[None, :], (P, WIN)).copy()
    bm2b = np.broadcast_to(np.asarray(inputs["bm2"], np.float32)[None, :], (P, OUT)).copy()

    in_maps = []
    for c in range(NCORES):
        m = {
            "xT": np.ascontiguousarray(x[c * SH:(c + 1) * SH, :].T),
            "W0": np.asarray(inputs["W0"], np.float32),
            "W1": np.asarray(inputs["W1"], np.float32),
            "W2": np.asarray(inputs["W2"], np.float32),
            "Wm1": np.asarray(inputs["Wm1"], np.float32),
            "Wm2": np.asarray(inputs["Wm2"], np.float32),
            "bm1": np.asarray(inputs["bm1"], np.float32).reshape(HID, 1),
            "bm2b": bm2b,
            "iota": iota,
        }
        for i in range(3):
            m[f"S{i}"] = Ss[i].reshape(HID, 1)
            m[f"T{i}"] = Ts[i].reshape(HID, 1)
        for s in range(2):
            A, dl_t, co_t = data[c][s]
            m[f"idx{s}"] = A
            m[f"dstl{s}"] = dl_t
            m[f"coef{s}"] = co_t
        in_maps.append(m)
    return in_maps


def _run(inputs, trace=False):
    sched, data = _preprocess(inputs["src"], inputs["dst"])
    nc = _build_program(sched)
    nc.compile()
    in_maps = _build_inputs(inputs, sched, data)
    res = run_bass_kernel_spmd(nc, in_maps, core_ids=list(range(NCORES)),
                               trace=trace)
    out = np.concatenate([res.results[c]["out"] for c in range(NCORES)], axis=0)
    return out, res


def kernel(**inputs) -> np.ndarray:
    out, _ = _run(inputs)
    return out


# revision 12
# speedup vs baseline: 1.1068x; 1.1068x over previous
"""GCN (3x GCNConv+BN+ReLU -> 2-layer MLP) on 8 Trainium2 NeuronCores.

Strategy (graph/data parallel, hardcoded for N=50000, E=800000):
  - Nodes sharded 6250/core.  Edges owned by dst core.  Self-loop terms are
    appended as explicit edges (coef = dinv^2) on the host.
  - Per layer: each core computes t = h @ W rows for its own nodes (PE),
    AllGather replicates the full [50000,128] table into every core's DRAM,
    then each core gathers t[src] rows for its edges with SWDGE dma_gather
    (int16 indices -> the table is addressed in two 25000-row halves),
    and scatter-adds into 125-node windows with a one-hot matmul trick:
        onehot[e, n] = (iota[n] == dst_local[e]) * coef[e]   (one DVE op)
        psum[f, n]  += gathered[e, f].T @ onehot[e, n]        (one PE matmul)
    BN+ReLU is fused into the PSUM eviction as a single ScalarE activation
    (per-feature scale/bias live on partitions).
  - Final MLP runs on local shards; output rows DMA'd out and concatenated
    on the host.

All graph preprocessing (degree, normalization coefs, edge sort/padding,
index layout for dma_gather) happens on the host in numpy; the edge
schedule is padded to be identical across all 8 cores (SPMD: one program).
"""

import math
import os

import numpy as np

import concourse.bass as bass
import concourse.bacc as bacc
import concourse.mybir as mybir
from concourse.tile import TileContext, add_dep_helper
from concourse.bass_utils import run_bass_kernel_spmd

# ----- problem constants (hardcoded; must match the grading problem) -----
N = 50000
E = 800000
IN_DIM = 64
HID = 128
OUT = 64
EPS = 1e-5
NCORES = 8
WIN = 125            # nodes per scatter window (PSUM free dim)
CHUNK = 1024         # edges per dma_gather call (8 groups of 128);
                     # 2048-idx dma_gather calls crash the exec unit
NSWQ = 4             # SWDGE queues; gather calls round-robin across Q7 pairs
P = 128              # partitions / edge-group size

F32 = mybir.dt.float32
I16 = mybir.dt.int16


def _derived():
    SH = N // NCORES
    NW = SH // WIN
    HALF = N // 2
    assert SH * NCORES == N and NW * WIN == SH and HALF * 2 == N
    return SH, NW, HALF


def _row_of_node(n):
    """DRAM table row of node n (table stores shard tiles in [p, i, f] order:
    local l = i*WIN + p  ->  row = rank*SH + p*NW + i)."""
    SH, NW, _ = _derived()
    r = n // SH
    l = n % SH
    return r * SH + (l % WIN) * NW + l // WIN


def _preprocess(src, dst):
    """Build the (identical-across-cores) edge schedule + per-core data."""
    SH, NW, HALF = _derived()
    src = np.asarray(src).astype(np.int64)
    dst = np.asarray(dst).astype(np.int64)

    deg = 1.0 + np.bincount(dst, minlength=N).astype(np.float64)
    dinv = (1.0 / np.sqrt(deg)).astype(np.float32)

    per_core = []
    counts = np.zeros((NCORES, NW, 2), np.int64)
    for c in range(NCORES):
        sel = (dst // SH) == c
        es = src[sel]
        ed = dst[sel]
        coef = (dinv[es] * dinv[ed]).astype(np.float32)
        nodes = np.arange(c * SH, (c + 1) * SH, dtype=np.int64)
        es = np.concatenate([es, nodes])
        ed = np.concatenate([ed, nodes])
        coef = np.concatenate([coef, (dinv[nodes] * dinv[nodes]).astype(np.float32)])
        l = ed - c * SH
        win = l // WIN
        nloc = l % WIN
        half = es // HALF
        order = np.lexsort((half, win))
        es, coef, win, nloc, half = (a[order] for a in (es, coef, win, nloc, half))
        np.add.at(counts[c], (win, half), 1)
        per_core.append((es, coef, win, nloc, half))

    # groups per (window, half): max over cores, in units of 128 edges
    G = np.ceil(counts.max(axis=0) / P).astype(np.int64)  # [NW, 2]
    G = np.maximum(G, 1)

    # per-half stream layouts (same for every core)
    stream_len = [int(G[:, s].sum()) * P for s in range(2)]
    n_groups = [int(G[:, s].sum()) for s in range(2)]
    # group j of stream s targets window gw[s][j]
    gw = [np.repeat(np.arange(NW), G[:, s]) for s in range(2)]

    row_of = _row_of_node(np.arange(N, dtype=np.int64))

    data = []
    for c in range(NCORES):
        es, coef, win, nloc, half = per_core[c]
        rows = row_of[es]
        streams = []
        for s in range(2):
            L = stream_len[s]
            idx_local = np.zeros(L, np.int64)
            co = np.zeros(L, np.float32)
            dl = np.zeros(L, np.float32)
            pos = 0
            for w in range(NW):
                m = (win == w) & (half == s)
                k = int(m.sum())
                cap = int(G[w, s]) * P
                assert k <= cap
                idx_local[pos:pos + k] = rows[m] - s * HALF
                co[pos:pos + k] = coef[m]
                dl[pos:pos + k] = nloc[m].astype(np.float32)
                # padding: idx 0 (valid row), coef 0 -> contributes nothing
                pos += cap
            assert pos == L
            assert idx_local.min() >= 0 and idx_local.max() < HALF <= 32768
            # idx tile [128, L/16]: chunk-local wrap (p = j%16, col = j//16),
            # replicated across the 8 groups of 16 partitions
            A = np.zeros((P, L // 16), np.int16)
            pos = 0
            while pos < L:
                cn = min(CHUNK, L - pos)
                blk = idx_local[pos:pos + cn].reshape(cn // 16, 16).T  # [16, cn/16]
                c0 = pos // 16
                for r in range(8):
                    A[r * 16:(r + 1) * 16, c0:c0 + cn // 16] = blk
                pos += cn
            ng = n_groups[s]
            dl_t = dl.reshape(ng, P).T.copy()          # [128, ng]
            co_t = co.reshape(ng, P).T.copy()          # [128, ng]
            streams.append((A, dl_t, co_t))
        data.append(streams)

    sched = {
        "G": G, "gw": gw, "stream_len": stream_len, "n_groups": n_groups,
        "dinv": dinv,
    }
    return sched, data


def _chunks_of(L):
    out = []
    pos = 0
    while pos < L:
        cn = min(CHUNK, L - pos)
        out.append((pos, cn))
        pos += cn
    return out


def _build_program(sched):
    SH, NW, HALF = _derived()
    G = sched["G"]
    stream_len = sched["stream_len"]
    n_groups = sched["n_groups"]

    nc = bacc.Bacc(trn_type="TRN2", num_devices=NCORES,
                   num_swdge_queues=NSWQ, dynamic_dma_scratch_size=16384 * NSWQ)
    f = F32

    # ---- I/O ----
    xT_d = nc.dram_tensor("xT", [IN_DIM, SH], f, kind="ExternalInput")
    W_d = [
        nc.dram_tensor("W0", [IN_DIM, HID], f, kind="ExternalInput"),
        nc.dram_tensor("W1", [HID, HID], f, kind="ExternalInput"),
        nc.dram_tensor("W2", [HID, HID], f, kind="ExternalInput"),
    ]
    S_d = [nc.dram_tensor(f"S{i}", [HID, 1], f, kind="ExternalInput") for i in range(3)]
    T_d = [nc.dram_tensor(f"T{i}", [HID, 1], f, kind="ExternalInput") for i in range(3)]
    Wm1_d = nc.dram_tensor("Wm1", [HID, HID], f, kind="ExternalInput")
    Wm2_d = nc.dram_tensor("Wm2", [HID, OUT], f, kind="ExternalInput")
    bm1_d = nc.dram_tensor("bm1", [HID, 1], f, kind="ExternalInput")
    bm2b_d = nc.dram_tensor("bm2b", [P, OUT], f, kind="ExternalInput")
    iota_d = nc.dram_tensor("iota", [P, WIN], f, kind="ExternalInput")
    idx_d = [nc.dram_tensor(f"idx{s}", [P, stream_len[s] // 16], I16,
                            kind="ExternalInput") for s in range(2)]
    dstl_d = [nc.dram_tensor(f"dstl{s}", [P, n_groups[s]], f,
                             kind="ExternalInput") for s in range(2)]
    coef_d = [nc.dram_tensor(f"coef{s}", [P, n_groups[s]], f,
                             kind="ExternalInput") for s in range(2)]

    shard_d = nc.dram_tensor("shard", [SH, HID], f, kind="Internal")
    tab_d = [nc.dram_tensor(f"tab{i}", [N, HID], f, kind="Internal")
             for i in range(2)]
    out_d = nc.dram_tensor("out", [SH, OUT], f, kind="ExternalOutput")

    rg = [list(range(NCORES))]
    relu = mybir.ActivationFunctionType.Relu
    copyf = mybir.ActivationFunctionType.Copy

    with TileContext(nc) as tc:
        with (
            tc.tile_pool(name="const", bufs=1) as constp,
            tc.tile_pool(name="hTp", bufs=2) as hTp,
            tc.tile_pool(name="g0p", bufs=3) as g0p,
            tc.tile_pool(name="g1p", bufs=3) as g1p,
            tc.tile_pool(name="ohp", bufs=6) as ohp,
            tc.tile_pool(name="rowp", bufs=4) as rowp,
            tc.tile_pool(name="psA", bufs=2, space="PSUM") as psA,
            tc.tile_pool(name="psB", bufs=2, space="PSUM") as psB,
        ):
            # ---- resident constants ----
            xT_sb = constp.tile([IN_DIM, SH], f, name="xT_sb")
            nc.sync.dma_start(xT_sb[:], xT_d[:])
            W_sb = []
            for i in range(3):
                t = constp.tile([W_d[i].shape[0], HID], f, name=f"W{i}_sb")
                nc.sync.dma_start(t[:], W_d[i][:])
                W_sb.append(t)
            Wm1_sb = constp.tile([HID, HID], f, name="Wm1_sb")
            nc.sync.dma_start(Wm1_sb[:], Wm1_d[:])
            Wm2_sb = constp.tile([HID, OUT], f, name="Wm2_sb")
            nc.sync.dma_start(Wm2_sb[:], Wm2_d[:])
            bm1_sb = constp.tile([HID, 1], f, name="bm1_sb")
            nc.sync.dma_start(bm1_sb[:], bm1_d[:])
            bm2b_sb = constp.tile([P, OUT], f, name="bm2b_sb")
            nc.sync.dma_start(bm2b_sb[:], bm2b_d[:])
            iota_sb = constp.tile([P, WIN], f, name="iota_sb")
            nc.sync.dma_start(iota_sb[:], iota_d[:])
            S_sb, T_sb = [], []
            for i in range(3):
                ts_ = constp.tile([HID, 1], f, name=f"S{i}_sb")
                nc.sync.dma_start(ts_[:], S_d[i][:])
                S_sb.append(ts_)
                tt_ = constp.tile([HID, 1], f, name=f"T{i}_sb")
                nc.sync.dma_start(tt_[:], T_d[i][:])
                T_sb.append(tt_)
            idx_sb, dstl_sb, coef_sb = [], [], []
            for s in range(2):
                ti = constp.tile([P, stream_len[s] // 16], I16, name=f"idx{s}_sb")
                nc.sync.dma_start(ti[:], idx_d[s][:])
                idx_sb.append(ti)
                td = constp.tile([P, n_groups[s]], f, name=f"dstl{s}_sb")
                nc.sync.dma_start(td[:], dstl_d[s][:])
                dstl_sb.append(td)
                tcf = constp.tile([P, n_groups[s]], f, name=f"coef{s}_sb")
                nc.sync.dma_start(tcf[:], coef_d[s][:])
                coef_sb.append(tcf)

            shard_v = shard_d[:].rearrange("(p i) a -> p i a", i=NW)  # [125,50,128]

            hT = None  # [128, SH] activations, transposed (feat on partitions)
            for layer in range(3):
                # ---- phase 1: shard rows of t = h @ W (node-tile i holds
                #      locals l = i*WIN + p; table row = p*NW + i) ----
                for i in range(NW):
                    if layer == 0:
                        lhsT = xT_sb[:, i * WIN:(i + 1) * WIN]
                    else:
                        lhsT = hT[:, i * WIN:(i + 1) * WIN]
                    ps = psB.tile([WIN, HID], f, tag="ph1", name="ps_ph1")
                    nc.tensor.matmul(ps[:], lhsT, W_sb[layer][:],
                                     start=True, stop=True)
                    row_sb = rowp.tile([WIN, HID], f, tag="row", name="row_sb")
                    nc.scalar.activation(row_sb[:], ps[:], copyf)
                    nc.sync.dma_start(shard_v[:, i, :], row_sb[:])

                tab = tab_d[layer % 2]
                cc = nc.gpsimd.collective_compute(
                    "AllGather", mybir.AluOpType.bypass, replica_groups=rg,
                    ins=[shard_d[:]], outs=[tab[:]],
                )

                # ---- phase 2: gather + one-hot scatter matmul ----
                hT_new = hTp.tile([HID, SH], f, tag="hT", name="hT_new")
                cur_chunk = [-1, -1]
                chunk_tile = [None, None]
                gpools = [g0p, g1p]
                gbase = [0, 0]  # group index base per stream
                chunk_list = [_chunks_of(stream_len[s]) for s in range(2)]

                for w in range(NW):
                    ps = psA.tile([HID, WIN], f, tag="agg", name="ps_agg")
                    ngw = int(G[w, 0] + G[w, 1])
                    k = 0
                    for s in range(2):
                        for j in range(gbase[s], gbase[s] + int(G[w, s])):
                            ck = (j * P) // CHUNK
                            if cur_chunk[s] != ck:
                                pos, cn = chunk_list[s][ck]
                                gt = gpools[s].tile(
                                    [P, cn // P, HID], f, tag=f"g{s}",
                                    name=f"gt{s}",
                                )
                                gi = nc.gpsimd.dma_gather(
                                    gt[:],
                                    tab[s * HALF:(s + 1) * HALF, :],
                                    idx_sb[s][:, pos // 16:(pos + cn) // 16],
                                    cn, cn, HID,
                                    queue_num=(s * NW + ck) % NSWQ,
                                )
                                add_dep_helper(gi.ins, cc.ins, True,
                                               "gather after allgather")
                                chunk_tile[s] = gt
                                cur_chunk[s] = ck
                            g_ap = chunk_tile[s][:, (j * P % CHUNK) // P, :]
                            oh = ohp.tile([P, WIN], f, tag="oh", name="oh")
                            nc.vector.tensor_scalar(
                                oh[:], iota_sb[:],
                                dstl_sb[s][:, j:j + 1],
                                coef_sb[s][:, j:j + 1],
                                mybir.AluOpType.is_equal,
                                mybir.AluOpType.mult,
                            )
                            nc.tensor.matmul(ps[:], g_ap, oh[:],
                                             start=(k == 0), stop=(k == ngw - 1))
                            k += 1
                    gbase[0] += int(G[w, 0])
                    gbase[1] += int(G[w, 1])
                    # fused BN+ReLU eviction
                    nc.scalar.activation(
                        hT_new[:, w * WIN:(w + 1) * WIN], ps[:], relu,
                        bias=T_sb[layer][:], scale=S_sb[layer][:],
                    )
                # reset per-layer chunk state
                hT = hT_new

            # ---- MLP ----
            z1T = hTp.tile([HID, SH], f, tag="hT", name="z1T")
            pos = 0
            while pos < SH:
                cn = min(512, SH - pos)
                ps = psB.tile([HID, 512], f, tag="mlp1", name="ps_mlp1")
                nc.tensor.matmul(ps[:, :cn], Wm1_sb[:], hT[:, pos:pos + cn],
                                 start=True, stop=True)
                nc.scalar.activation(z1T[:, pos:pos + cn], ps[:, :cn], relu,
                                     bias=bm1_sb[:])
                pos += cn
            for i in range(NW):
                ps2 = psB.tile([WIN, OUT], f, tag="mlp2", name="ps_mlp2")
                nc.tensor.matmul(ps2[:], z1T[:, i * WIN:(i + 1) * WIN],
                                 Wm2_sb[:], start=True, stop=True)
                ot = rowp.tile([WIN, OUT], f, tag="ot", name="ot")
                nc.vector.tensor_tensor(ot[:], ps2[:], bm2b_sb[:WIN, :],
                                        op=mybir.AluOpType.add)
                nc.sync.dma_start(out_d[i * WIN:(i + 1) * WIN, :], ot[:])

    return nc


def _build_inputs(inputs, sched, data):
    """Per-core in_maps."""
    SH, NW, HALF = _derived()
    x = np.asarray(inputs["x"], np.float32)

    def bnfold(g, be, m, v, b):
        S = (np.asarray(g, np.float32)
             / np.sqrt(np.asarray(v, np.float32) + np.float32(EPS)))
        T = np.asarray(be, np.float32) + (np.asarray(b, np.float32)
                                          - np.asarray(m, np.float32)) * S
        return S.astype(np.float32), T.astype(np.float32)

    S0, T0 = bnfold(inputs["g0"], inputs["be0"], inputs["m0"], inputs["v0"], inputs["b0"])
    S1, T1 = bnfold(inputs["g1"], inputs["be1"], inputs["m1"], inputs["v1"], inputs["b1"])
    S2, T2 = bnfold(inputs["g2"], inputs["be2"], inputs["m2"], inputs["v2"], inputs["b2"])
    Ss = [S0, S1, S2]
    Ts = [T0, T1, T2]

    iota = np.broadcast_to(np.arange(WIN, dtype=np.float32)[None, :], (P, WIN)).copy()
    bm2b = np.broadcast_to(np.asarray(inputs["bm2"], np.float32)[None, :], (P, OUT)).copy()

    in_maps = []
    for c in range(NCORES):
        m = {
            "xT": np.ascontiguousarray(x[c * SH:(c + 1) * SH, :].T),
            "W0": np.asarray(inputs["W0"], np.float32),
            "W1": np.asarray(inputs["W1"], np.float32),
            "W2": np.asarray(inputs["W2"], np.float32),
            "Wm1": np.asarray(inputs["Wm1"], np.float32),
            "Wm2": np.asarray(inputs["Wm2"], np.float32),
            "bm1": np.asarray(inputs["bm1"], np.float32).reshape(HID, 1),
            "bm2b": bm2b,
            "iota": iota,
        }
        for i in range(3):
            m[f"S{i}"] = Ss[i].reshape(HID, 1)
            m[f"T{i}"] = Ts[i].reshape(HID, 1)
        for s in range(2):
            A, dl_t, co_t = data[c][s]
            m[f"idx{s}"] = A
            m[f"dstl{s}"] = dl_t
            m[f"coef{s}"] = co_t
        in_maps.append(m)
    return in_maps


def _run(inputs, trace=False):
    sched, data = _preprocess(inputs["src"], inputs["dst"])
    nc = _build_program(sched)
    nc.compile()
    in_maps = _build_inputs(inputs, sched, data)
    res = run_bass_kernel_spmd(nc, in_maps, core_ids=list(range(NCORES)),
                               trace=trace)
    out = np.concatenate([res.results[c]["out"] for c in range(NCORES)], axis=0)
    return out, res


def kernel(**inputs) -> np.ndarray:
    out, _ = _run(inputs)
    return out
